# revision 33
# baseline (speedup 1.0000x reference)
"""Causal self-attention (RoPE, 16 heads) on 8 Trainium2 NeuronCores.

Sharding: data parallel over batch (2) x tensor parallel over head groups
(16 heads -> 4 groups of 4). Core c handles batch c//4, head group c%4.
Each core computes q/k/v projections for its 4 heads, RoPE, causal
softmax(q k^T / sqrt(d)) v, and its slice of the output projection; the
host sums the 4 tensor-parallel partials per batch.

Layouts (per core):
  xT [1024 D, 2048 S]   q/k transposed [256 ch, 2048 S] (head dim on
  partitions, so scores need no transposes), v natural [2048 S, 4, 64+1]
  with a ones column so attn@v also produces the softmax denominators.
  Scores are computed transposed S[k, q]; attn@v uses the exp tiles as
  the stationary operand giving o in natural [q, ch] layout, where the
  denominator lands in a psum column -> per-partition reciprocal +
  tensor_scalar normalize. o is then PE-transposed back to [ch, q] for
  the output projection. fp16 operands, fp32 psum accumulation.

Schedule: weights are DMAd first (small), then xt arrives in column
halves interleaved across two issue queues at the same rate the
projection chains consume them, so the PE starts ~7.5us in and stays
dense. Attention runs qc-outer/th-inner; each block's first attn@v and
the previous block's normalize/transpose/output-projection are deferred
into the next block's score stream so the PE never waits on the Vector
engine at block boundaries. PSUM accumulators are zeroed through the
bank's start=True write semantics (first write per bank clears it).
"""
import numpy as np

import concourse.bass as bass
import concourse.mybir as mybir
import concourse.tile as tile
from concourse.vector_clock import ScopedClock
from concourse.bass_utils import run_bass_kernel_spmd

F32 = mybir.dt.float32
F16 = mybir.dt.float16

D_MODEL = 1024
N_HEADS = 16
HEAD_DIM = 64
SEQ = 2048
BATCH = 2
N_CORES = 8
HEADS_PER_CORE = 4
GROUPS = 4
CH = HEADS_PER_CORE * HEAD_DIM  # 256

MAX_WAITS = 1


def _cap_waits(nc: bass.Bass, cap: int):
    """walrus here only accepts `cap` sem waits per instruction; hoist the
    overflow onto same-engine nops inserted just before."""
    nid = [0]

    def mknop(engine, waits):
        nid[0] += 1
        n = mybir.InstNoOp(name=f"I-waitcap-{nid[0]}", ins=[], outs=[])
        n.engine = engine
        n.sync_info = mybir.SyncInfo(on_wait=list(waits), on_update=[])
        return n

    for fn in nc.m.functions:
        for bb in fn.blocks:
            out = []
            changed = False
            for ins in bb.instructions:
                si = ins.sync_info
                w = list(si.on_wait) if si and si.on_wait else []
                if len(w) > cap:
                    changed = True
                    keep = w[-cap:]
                    rest = w[: len(w) - cap]
                    eng = ins.engine
                    if eng == mybir.EngineType.Unassigned:
                        eng = mybir.EngineType.SP
                    for i in range(0, len(rest), cap):
                        out.append(mknop(eng, rest[i : i + cap]))
                    si.on_wait = keep
                out.append(ins)
            if changed:
                bb.instructions = out


class KTileContext(tile.TileContext):
    def _drain_and_barrier(self, tick_clock, wait_clock):
        drain_inst = self.nc.sync.drain()
        wait_clock.add_sem_waits(
            drain_inst.ins, ScopedClock({None: tick_clock.global_clock})
        )
        si = drain_inst.ins.sync_info
        w = si.on_wait if si else None
        if w and len(w) > 1:
            si.on_wait = []
            for sw in w:
                n2 = self.nc.sync.nop()
                if n2.ins.sync_info is None:
                    n2.ins.sync_info = mybir.SyncInfo(on_wait=[sw], on_update=[])
                else:
                    n2.ins.sync_info.on_wait = [sw]
            self.nc.sync.drain()
        self.nc.all_engine_barrier()
        assert self.sems is not None
        popped = self.nc._tile_sem_poison_stack.pop()
        assert popped is self._sem_poison
        self.nc.clear_and_free_semaphores(list(self.sems.allocated().values()))
        self.nc.all_engine_barrier()

    def __exit__(self, exc_type, exc_value, traceback):
        r = super().__exit__(exc_type, exc_value, traceback)
        if exc_type is None:
            _cap_waits(self.nc, MAX_WAITS)
        return r


def build_program() -> bass.Bass:
    nc = bass.Bass()

    xt_d = nc.dram_tensor("xt", [D_MODEL, SEQ], F16, kind="ExternalInput")
    wq_d = nc.dram_tensor("wq", [D_MODEL, CH], F16, kind="ExternalInput")
    wk_d = nc.dram_tensor("wk", [D_MODEL, CH], F16, kind="ExternalInput")
    wv_d = nc.dram_tensor("wv", [D_MODEL, CH], F16, kind="ExternalInput")
    wo_d = nc.dram_tensor("wo", [CH, D_MODEL], F16, kind="ExternalInput")
    cos_d = nc.dram_tensor("cos2", [64, SEQ], F16, kind="ExternalInput")
    sin_d = nc.dram_tensor("sin2", [64, SEQ], F16, kind="ExternalInput")
    msk_d = nc.dram_tensor("msk", [128, 384], F16, kind="ExternalInput")
    rot_d = nc.dram_tensor("rot", [128, 128], F16, kind="ExternalInput")
    idn_d = nc.dram_tensor("idn", [128, 128], F16, kind="ExternalInput")
    out_d = nc.dram_tensor("out", [SEQ, D_MODEL], F16, kind="ExternalOutput")

    NQ = SEQ // 512       # 4 q chunks of 512
    NROW = SEQ // 128     # 16 row chunks / q tiles
    KD = D_MODEL // 128   # 8 contraction chunks
    # xt tiles land in this order (k0..3 on sync queue, k4..7 on gpsimd,
    # issued in parallel) -- projection chains consume in arrival order.
    KORDER = [0, 4, 1, 5, 2, 6, 3, 7]

    with KTileContext(nc) as tc, nc.allow_low_precision(reason="fp16 pipeline"):
        with (
            tc.tile_pool(name="wgt", bufs=1) as wgt,
            tc.tile_pool(name="tabs", bufs=1) as tabs,
            tc.tile_pool(name="qk", bufs=1) as qkp,
            tc.tile_pool(name="vp", bufs=1) as vp,
            tc.tile_pool(name="xt", bufs=1) as xtp,
            tc.tile_pool(name="op", bufs=1) as op,
        ):
            wq_sb = [wgt.tile([128, CH], F16, name=f"wq{k}", tag=f"wq{k}") for k in range(KD)]
            wk_sb = [wgt.tile([128, CH], F16, name=f"wk{k}", tag=f"wk{k}") for k in range(KD)]
            wv_sb = [wgt.tile([128, CH], F16, name=f"wv{k}", tag=f"wv{k}") for k in range(KD)]
            cos_sb = tabs.tile([128, SEQ], F16, tag="cos")
            sin_sb = tabs.tile([128, SEQ], F16, tag="sin")
            rot_sb = tabs.tile([128, 128], F16, tag="rot")
            idn_sb = tabs.tile([128, 128], F16, tag="idn")
            msk_sb = tabs.tile([128, 384], F16, tag="msk")
            q_sb = [qkp.tile([128, SEQ], F16, name=f"q{t}", tag=f"q{t}") for t in range(2)]
            k_sb = [qkp.tile([128, SEQ], F16, name=f"k{t}", tag=f"k{t}") for t in range(2)]
            v_sb = [vp.tile([128, HEADS_PER_CORE, 65], F16, name=f"v{r}", tag=f"v{r}")
                    for r in range(NROW)]
            xt_sb = [xtp.tile([128, SEQ], F16, name=f"xt{k}", tag=f"xt{k}") for k in range(KD)]
            oT = [op.tile([128, SEQ], F16, name=f"oT{t}", tag=f"oT{t}") for t in range(2)]
            wo_sb = [op.tile([128, D_MODEL], F16, name=f"wo{k}", tag=f"wo{k}")
                     for k in range(2)]

            # ---- DMA issue. Weights first (small), then xt column halves
            # interleaved so chains can consume at arrival rate; tables,
            # wv and wo only after all xt (they are needed much later).
            HALF = SEQ // 2
            nc.sync.dma_start(out=wq_sb[0][:], in_=wq_d[0:128, :])
            nc.sync.dma_start(out=wq_sb[4][:], in_=wq_d[512:640, :])
            for i in range(4):
                nc.sync.dma_start(out=xt_sb[i][:, 0:HALF],
                                  in_=xt_d[i * 128:(i + 1) * 128, 0:HALF])
                if i < 3:
                    nc.sync.dma_start(out=wq_sb[i + 1][:],
                                      in_=wq_d[(i + 1) * 128:(i + 2) * 128, :])
                    nc.sync.dma_start(out=wq_sb[i + 5][:],
                                      in_=wq_d[(i + 5) * 128:(i + 6) * 128, :])
            for i in range(4):
                nc.sync.dma_start(out=xt_sb[i][:, HALF:SEQ],
                                  in_=xt_d[i * 128:(i + 1) * 128, HALF:SEQ])
            nc.gpsimd.dma_start(out=wk_sb[0][:], in_=wk_d[0:128, :])
            nc.gpsimd.dma_start(out=wk_sb[4][:], in_=wk_d[512:640, :])
            nc.gpsimd.dma_start(out=rot_sb[:], in_=rot_d[:])
            for i in range(4):
                k = i + 4
                nc.gpsimd.dma_start(out=xt_sb[k][:, 0:HALF],
                                    in_=xt_d[k * 128:(k + 1) * 128, 0:HALF])
                if i < 3:
                    nc.gpsimd.dma_start(out=wk_sb[i + 1][:],
                                        in_=wk_d[(i + 1) * 128:(i + 2) * 128, :])
                    nc.gpsimd.dma_start(out=wk_sb[i + 5][:],
                                        in_=wk_d[(i + 5) * 128:(i + 6) * 128, :])
            for i in range(4):
                k = i + 4
                nc.gpsimd.dma_start(out=xt_sb[k][:, HALF:SEQ],
                                    in_=xt_d[k * 128:(k + 1) * 128, HALF:SEQ])
            nc.gpsimd.dma_start(out=cos_sb[0:64, :], in_=cos_d[:])
            nc.gpsimd.dma_start(out=cos_sb[64:128, :], in_=cos_d[:])
            nc.gpsimd.dma_start(out=sin_sb[0:64, :], in_=sin_d[:])
            nc.gpsimd.dma_start(out=sin_sb[64:128, :], in_=sin_d[:])
            for k in range(KD):
                nc.gpsimd.dma_start(out=wv_sb[k][:], in_=wv_d[k * 128:(k + 1) * 128, :])
            nc.gpsimd.dma_start(out=idn_sb[:], in_=idn_d[:])
            nc.gpsimd.dma_start(out=msk_sb[:], in_=msk_d[:])
            for k in range(2):
                nc.gpsimd.dma_start(out=wo_sb[k][:], in_=wo_d[k * 128:(k + 1) * 128, :])

            for r in range(NROW):
                nc.vector.memset(v_sb[r][:, :, 64:65], 1.0)

            # ---------------- phase 1: projections + RoPE ----------------
            with (
                tc.tile_pool(name="praw", bufs=2) as praw,
                tc.tile_pool(name="pp", bufs=1, space="PSUM") as pp,
                tc.tile_pool(name="rp", bufs=2, space="PSUM") as rp,
            ):
                def rope(m, which, raw, dst):
                    for n in range(NQ):
                        sl = slice(n * 512, (n + 1) * 512)
                        pr = rp.tile([128, 512], F32, name=f"pr{m}{which}{n}", tag="prot")
                        nc.tensor.matmul(pr[:], rot_sb[:], raw[:, sl],
                                         start=True, stop=True)
                        t1 = praw.tile([128, 512], F16, name=f"t1{m}{which}{n}", tag="t1")
                        nc.vector.tensor_mul(t1[:], raw[:, sl], cos_sb[:, sl])
                        t2 = praw.tile([128, 512], F16, name=f"t2{m}{which}{n}", tag="t2")
                        nc.vector.tensor_mul(t2[:], pr[:], sin_sb[:, sl])
                        nc.vector.tensor_add(dst[:, sl], t1[:], t2[:])

                # --- head pair 0: two waves of 4 interleaved chains, each
                # wave consuming one xt column-half in arrival order.
                raw_q0 = praw.tile([128, SEQ], F16, name="rawq0", tag="raw")
                raw_k0 = praw.tile([128, SEQ], F16, name="rawk0", tag="raw")
                for half in range(2):
                    chains = []
                    for which in ("q", "k"):
                        for j in range(2):
                            n = 2 * half + j
                            ps = pp.tile([128, 512], F32,
                                         name=f"ps0{which}{n}", tag=f"p{which}{j}")
                            chains.append((which, n, ps))
                    for i, k in enumerate(KORDER):
                        st, sp = (i == 0), (i == KD - 1)
                        for which, n, ps in chains:
                            w_sb = wq_sb if which == "q" else wk_sb
                            nc.tensor.matmul(
                                ps[:], w_sb[k][:, 0:128],
                                xt_sb[k][:, n * 512:(n + 1) * 512],
                                start=st, stop=sp)
                    for which, n, ps in chains:
                        raw = raw_q0 if which == "q" else raw_k0
                        nc.scalar.copy(raw[:, n * 512:(n + 1) * 512], ps[:])
                rope(0, "q", raw_q0, q_sb[0])
                rope(0, "k", raw_k0, k_sb[0])

                # --- head pair 1: plain chains (xt resident)
                for which, w_sb, dst in (("q", wq_sb, q_sb), ("k", wk_sb, k_sb)):
                    raw = praw.tile([128, SEQ], F16, name=f"raw1{which}", tag="raw")
                    for n in range(NQ):
                        ps = pp.tile([128, 512], F32, name=f"ps1{which}{n}",
                                     tag=f"p{which}{n % 2}")
                        for i, k in enumerate(KORDER):
                            nc.tensor.matmul(
                                ps[:], w_sb[k][:, 128:256],
                                xt_sb[k][:, n * 512:(n + 1) * 512],
                                start=(i == 0), stop=(i == KD - 1))
                        nc.scalar.copy(raw[:, n * 512:(n + 1) * 512], ps[:])
                    rope(1, which, raw, dst[1])

                # --- v projection rows 0..3 (rest interleaved into attn)
                for r in range(4):
                    tagn = ["pq0", "pq1", "pk0", "pk1"][r % 4]
                    ps = pp.tile([128, HEADS_PER_CORE, 64], F32,
                                 name=f"pv{r}", tag=tagn)
                    for i, k in enumerate(KORDER):
                        nc.tensor.matmul(
                            ps[:].rearrange("p a b -> p (a b)"),
                            xt_sb[k][:, r * 128:(r + 1) * 128], wv_sb[k][:],
                            start=(i == 0), stop=(i == KD - 1))
                    nc.scalar.copy(v_sb[r][:, :, 0:64], ps[:])

            # ---------------- phase 2: attention + output projection ----
            with (
                tc.tile_pool(name="se", bufs=5) as sep,
                tc.tile_pool(name="nrm", bufs=2) as nrm,
                tc.tile_pool(name="onat", bufs=8) as onp,
                tc.tile_pool(name="outp", bufs=3) as outp,
                tc.tile_pool(name="pss", bufs=2, space="PSUM") as pss,
                tc.tile_pool(name="pop", bufs=2, space="PSUM") as pop,
                tc.tile_pool(name="pmisc", bufs=2, space="PSUM") as pmisc,
            ):
                # v-projection rows 4..15 as PE filler units (two halves
                # each) popped into the attention stream: they keep the PE
                # dense through the exp-bound stretches and their psum
                # shares the pmisc ring.
                vfill = []
                for r in range(4, NROW):
                    def vch(r=r):
                        ps = pmisc.tile([128, HEADS_PER_CORE, 64], F32,
                                        name=f"pv{r}", tag="pm")
                        for i, k in enumerate(KORDER):
                            nc.tensor.matmul(
                                ps[:].rearrange("p a b -> p (a b)"),
                                xt_sb[k][:, r * 128:(r + 1) * 128], wv_sb[k][:],
                                start=(i == 0), stop=(i == KD - 1))
                        nc.vector.tensor_copy(v_sb[r][:, :, 0:64], ps[:])

                    vfill.append(("v", r, vch))

                def outproj_unit(qt):
                    def emit():
                        for nn in range(2):
                            pf = pmisc.tile([128, 512], F32,
                                            name=f"pf{qt}_{nn}", tag="pm")
                            for kk in range(2):
                                nc.tensor.matmul(
                                    pf[:],
                                    oT[kk][:, qt * 128:(qt + 1) * 128],
                                    wo_sb[kk][:, nn * 512:(nn + 1) * 512],
                                    start=(kk == 0), stop=(kk == 1))
                            ob = outp.tile([128, 512], F16,
                                           name=f"ob{qt}_{nn}", tag="ob")
                            nc.vector.tensor_copy(ob[:], pf[:])
                            nc.sync.dma_start(
                                out=out_d[qt * 128:(qt + 1) * 128,
                                          nn * 512:(nn + 1) * 512],
                                in_=ob[:])
                    return emit

                def fin_piece(qc, th, po, qt4):
                    # per-q-tile finalize for the last block: po[:, qt4] is
                    # complete once av(4qc+qt4) stopped, so normalize,
                    # transpose and project it while later kts still run
                    gq = 4 * qc + qt4
                    onat_t = onp.tile([128, 2, 64], F16,
                                      name=f"onp{th}_{gq}", tag="onat")
                    for hh in range(2):
                        rcol = nrm.tile([128, 1, 1], F32,
                                        name=f"rcp{th}{gq}{hh}", tag=f"rcp{hh}")
                        nc.vector.reciprocal(rcol[:], po[hh][:, qt4:qt4 + 1, 64:65])
                        nc.vector.tensor_scalar_mul(
                            onat_t[:, hh, :], po[hh][:, qt4, 0:64],
                            rcol[:, 0, :])
                    pt = pmisc.tile([128, 128], F16, name=f"ptl{gq}", tag="pm")
                    nc.tensor.transpose(
                        pt[:], onat_t[:].rearrange("p a b -> p (a b)"), idn_sb[:])
                    nc.vector.tensor_copy(
                        oT[th][:, gq * 128:(gq + 1) * 128], pt[:])
                    outproj_unit(gq)()

                def finalize(qc, th, po, fuse_outproj=False):
                    onat_tiles = []

                    def emit_norm():
                        for qt4 in range(4):
                            onat_tiles.append(
                                onp.tile([128, 2, 64], F16,
                                         name=f"on{th}_{4 * qc + qt4}",
                                         tag="onat"))
                        rcols = []
                        for hh in range(2):
                            rcol = nrm.tile([128, 4, 1], F32,
                                            name=f"rc{th}{qc}{hh}", tag=f"rcol{hh}")
                            nc.vector.reciprocal(rcol[:], po[hh][:, :, 64:65])
                            rcols.append(rcol)
                        for qt4 in range(4):
                            for hh in range(2):
                                nc.vector.tensor_scalar_mul(
                                    onat_tiles[qt4][:, hh, :],
                                    po[hh][:, qt4, 0:64],
                                    rcols[hh][:, qt4, :])

                    def emit_tr():
                        for qt4 in range(4):
                            gq = 4 * qc + qt4
                            pt = pmisc.tile([128, 128], F16,
                                            name=f"pt{th}_{gq}", tag="pm")
                            nc.tensor.transpose(
                                pt[:],
                                onat_tiles[qt4][:].rearrange("p a b -> p (a b)"),
                                idn_sb[:])
                            nc.vector.tensor_copy(
                                oT[th][:, gq * 128:(gq + 1) * 128], pt[:])
                            if fuse_outproj:
                                outproj_unit(gq)()
                    return emit_norm, emit_tr

                # Unified PE filler queue: v-projection chains (deadline:
                # v_sb[r] must be emitted before any av that reads it, since
                # the PE executes in order) and output projections (needed
                # only before the end).  Opportunistic pops spread the units
                # through the exp-bound stretches; deadline pops guarantee
                # correctness.
                fillq = list(vfill)
                AV_LAG = 1

                def drain_vdeadline(row):
                    while fillq and fillq[0][0] == "v" and fillq[0][1] <= row:
                        fillq.pop(0)[2]()

                pend_fin = None
                for qc in range(NQ):              # q chunk of 512
                    qs0 = qc * 512
                    nkt = 4 * qc + 4              # causal k tiles
                    for th in range(2):           # head pair
                        last = (qc == NQ - 1 and th == 1)
                        po = [pop.tile([128, HEADS_PER_CORE, 65], F32,
                                       name=f"po{th}_{qc}_{hh}", tag="po")
                              for hh in range(2)]
                        av_q = []
                        for kt in range(nkt):
                            rel = kt - 4 * qc
                            c0 = max(rel, 0) * 128
                            ps = pss.tile([128, 2, 512], F32,
                                          name=f"ps{th}_{qc}_{kt}", tag="ps")
                            for hh in range(2):
                                b0 = 64 * hh
                                nc.tensor.matmul(
                                    ps[:, hh, c0:512],
                                    k_sb[th][b0:b0 + 64, kt * 128:(kt + 1) * 128],
                                    q_sb[th][b0:b0 + 64, qs0 + c0:qs0 + 512],
                                    start=True, stop=True)
                            s = sep.tile([128, 2, 512], F16,
                                         name=f"s{th}_{qc}_{kt}", tag="se")
                            nc.scalar.activation(
                                s[:, :, c0:512], ps[:, :, c0:512],
                                mybir.ActivationFunctionType.Exp, scale=0.125)
                            if rel >= 0:
                                # gpsimd: all-SBUF op, keeps the mask off
                                # the (busier, latency-critical) DVE queue
                                for hh in range(2):
                                    nc.gpsimd.tensor_mul(
                                        s[:, hh, c0:c0 + 128],
                                        s[:, hh, c0:c0 + 128],
                                        msk_sb[:, 0:128])

                            def av(kt=kt, s=s, po=po, qc=qc, th=th):
                                # first write into each po bank clears it
                                # (start=True zero-pends the whole bank;
                                # later writes then land fresh / accumulate)
                                for hh in range(2):
                                    head = 2 * th + hh
                                    for qt4 in range(4):
                                        gq = 4 * qc + qt4
                                        if kt > gq:
                                            continue
                                        nc.tensor.matmul(
                                            po[hh][:, qt4, :],
                                            s[:, hh, qt4 * 128:(qt4 + 1) * 128],
                                            v_sb[kt][:, head, :],
                                            start=(kt == 0 and qt4 == 0),
                                            stop=(kt == gq),
                                            skip_group_check=True)

                            if kt == 1 and pend_fin is not None:
                                # DVE normalize first, a PE filler to cover
                                # its latency, then the PE transposes
                                pend_fin[0]()
                                # at th0 blocks the pending fin is th1's:
                                # an outproj unit here could read oT columns
                                # the transpose below hasn't written yet
                                if fillq and (fillq[0][0] == "v" or th == 1):
                                    fillq.pop(0)[2]()
                                pend_fin[1]()
                                pend_fin = None
                            drain_vdeadline(4 * qc + kt + 2)
                            if kt >= 2 and kt % 3 != 2 and fillq:
                                fillq.pop(0)[2]()
                            # attn@v lags one kt behind its scores so it
                            # never waits on the exp/mask chain
                            if len(av_q) == AV_LAG:
                                av_q.pop(0)()
                                if last and kt - AV_LAG >= 4 * qc:
                                    fin_piece(qc, th, po, kt - AV_LAG - 4 * qc)
                            av_q.append(av)
                        for i, a in enumerate(av_q):
                            a()
                            if last and nkt - len(av_q) + i >= 4 * qc:
                                fin_piece(qc, th, po, nkt - len(av_q) + i - 4 * qc)
                        av_q = []
                        if not last:
                            pend_fin = finalize(qc, th, po)
                        if th == 1 and not last:
                            for qt4 in range(4):
                                fillq.append(("o", 10 ** 6,
                                              outproj_unit(4 * qc + qt4)))
                if pend_fin is not None:
                    pend_fin[0]()
                    pend_fin[1]()
                for u in fillq:
                    u[2]()
    return nc


_PROGRAM_CACHE = {}


def _get_program():
    if "nc" not in _PROGRAM_CACHE:
        _PROGRAM_CACHE["nc"] = build_program()
    return _PROGRAM_CACHE["nc"]


def _host_inputs(x, cos, sin, Wq, Wk, Wv, Wo):
    f16 = np.float16
    cosT = np.ascontiguousarray(cos.T).astype(f16)
    sinT = np.ascontiguousarray(sin.T).astype(f16)

    R = np.zeros((HEAD_DIM, HEAD_DIM), np.float32)
    R[np.arange(32), np.arange(32) + 32] = -1.0
    R[np.arange(32) + 32, np.arange(32)] = 1.0
    RT = R.T
    rot = np.zeros((128, 128), np.float32)
    rot[0:64, 0:64] = RT
    rot[64:128, 64:128] = RT
    rot = rot.astype(f16)

    msk = np.zeros((128, 384), np.float32)
    p = np.arange(128)[:, None]
    f = np.arange(128)[None, :]
    msk[:, 0:128] = (p - f <= 0)          # triangular block; cols 128: zeros
    msk = msk.astype(f16)

    idn = np.eye(128, dtype=f16)

    in_maps = []
    for c in range(N_CORES):
        b, g = divmod(c, GROUPS)
        rows = slice(g * CH, (g + 1) * CH)
        in_maps.append({
            "xt": np.ascontiguousarray(x[b].T).astype(f16),
            "wq": np.ascontiguousarray(Wq[rows, :].T).astype(f16),
            "wk": np.ascontiguousarray(Wk[rows, :].T).astype(f16),
            "wv": np.ascontiguousarray(Wv[rows, :].T).astype(f16),
            "wo": np.ascontiguousarray(Wo[:, rows].T).astype(f16),
            "cos2": cosT, "sin2": sinT, "msk": msk, "rot": rot, "idn": idn,
        })
    return in_maps


def kernel(x, cos, sin, Wq, Wk, Wv, Wo, _trace=False, _trace_kwargs=None):
    nc = _get_program()
    in_maps = _host_inputs(x, cos, sin, Wq, Wk, Wv, Wo)
    kw = {}
    if _trace:
        kw["trace"] = True
        if _trace_kwargs:
            kw.update(_trace_kwargs)
    res = run_bass_kernel_spmd(nc, in_maps, list(range(N_CORES)), **kw)
    out = np.zeros((BATCH, SEQ, D_MODEL), np.float32)
    for c in range(N_CORES):
        b = c // GROUPS
        out[b] += res.results[c]["out"].astype(np.float32)
    kernel.last_result = res
    return out


# revision 35
# speedup vs baseline: 1.2322x; 1.2322x over previous
"""Causal self-attention (RoPE, 16 heads) on 8 Trainium2 NeuronCores.

Sharding: data parallel over batch (2) x tensor parallel over head groups
(16 heads -> 4 groups of 4). Core c handles batch c//4, head group c%4.
Each core computes q/k/v projections for its 4 heads, RoPE, causal
softmax(q k^T / sqrt(d)) v, and its slice of the output projection; the
host sums the 4 tensor-parallel partials per batch.

Layouts (per core):
  xT [1024 D, 2048 S]   q/k transposed [256 ch, 2048 S] (head dim on
  partitions, so scores need no transposes), v natural [2048 S, 4, 64+1]
  with a ones column so attn@v also produces the softmax denominators.
  Scores are computed transposed S[k, q]; attn@v uses the exp tiles as
  the stationary operand giving o in natural [q, ch] layout, where the
  denominator lands in a psum column -> per-partition reciprocal +
  tensor_scalar normalize. o is then PE-transposed back to [ch, q] for
  the output projection. fp16 operands, fp32 psum accumulation.

Schedule: weights are DMAd first (small), then xt arrives in column
halves interleaved across two issue queues at the same rate the
projection chains consume them, so the PE starts ~7.5us in and stays
dense. Attention runs qc-outer/th-inner; each block's first attn@v and
the previous block's normalize/transpose/output-projection are deferred
into the next block's score stream so the PE never waits on the Vector
engine at block boundaries. PSUM accumulators are zeroed through the
bank's start=True write semantics (first write per bank clears it).
"""
import numpy as np

import concourse.bass as bass
import concourse.mybir as mybir
import concourse.tile as tile
from concourse.vector_clock import ScopedClock
from concourse.bass_utils import run_bass_kernel_spmd

F32 = mybir.dt.float32
F16 = mybir.dt.float16

D_MODEL = 1024
N_HEADS = 16
HEAD_DIM = 64
SEQ = 2048
BATCH = 2
N_CORES = 8
HEADS_PER_CORE = 4
GROUPS = 4
CH = HEADS_PER_CORE * HEAD_DIM  # 256

MAX_WAITS = 1


def _cap_waits(nc: bass.Bass, cap: int):
    """walrus here only accepts `cap` sem waits per instruction; hoist the
    overflow onto same-engine nops inserted just before."""
    nid = [0]

    def mknop(engine, waits):
        nid[0] += 1
        n = mybir.InstNoOp(name=f"I-waitcap-{nid[0]}", ins=[], outs=[])
        n.engine = engine
        n.sync_info = mybir.SyncInfo(on_wait=list(waits), on_update=[])
        return n

    for fn in nc.m.functions:
        for bb in fn.blocks:
            out = []
            changed = False
            for ins in bb.instructions:
                si = ins.sync_info
                w = list(si.on_wait) if si and si.on_wait else []
                if len(w) > cap:
                    changed = True
                    keep = w[-cap:]
                    rest = w[: len(w) - cap]
                    eng = ins.engine
                    if eng == mybir.EngineType.Unassigned:
                        eng = mybir.EngineType.SP
                    for i in range(0, len(rest), cap):
                        out.append(mknop(eng, rest[i : i + cap]))
                    si.on_wait = keep
                out.append(ins)
            if changed:
                bb.instructions = out


class KTileContext(tile.TileContext):
    def _drain_and_barrier(self, tick_clock, wait_clock):
        drain_inst = self.nc.sync.drain()
        wait_clock.add_sem_waits(
            drain_inst.ins, ScopedClock({None: tick_clock.global_clock})
        )
        si = drain_inst.ins.sync_info
        w = si.on_wait if si else None
        if w and len(w) > 1:
            si.on_wait = []
            for sw in w:
                n2 = self.nc.sync.nop()
                if n2.ins.sync_info is None:
                    n2.ins.sync_info = mybir.SyncInfo(on_wait=[sw], on_update=[])
                else:
                    n2.ins.sync_info.on_wait = [sw]
            self.nc.sync.drain()
        self.nc.all_engine_barrier()
        assert self.sems is not None
        popped = self.nc._tile_sem_poison_stack.pop()
        assert popped is self._sem_poison
        self.nc.clear_and_free_semaphores(list(self.sems.allocated().values()))
        self.nc.all_engine_barrier()

    def __exit__(self, exc_type, exc_value, traceback):
        r = super().__exit__(exc_type, exc_value, traceback)
        if exc_type is None:
            _cap_waits(self.nc, MAX_WAITS)
        return r


def build_program() -> bass.Bass:
    nc = bass.Bass()

    xt_d = nc.dram_tensor("xt", [D_MODEL, SEQ], F16, kind="ExternalInput")
    wq_d = nc.dram_tensor("wq", [D_MODEL, CH], F16, kind="ExternalInput")
    wk_d = nc.dram_tensor("wk", [D_MODEL, CH], F16, kind="ExternalInput")
    wv_d = nc.dram_tensor("wv", [D_MODEL, CH], F16, kind="ExternalInput")
    wo_d = nc.dram_tensor("wo", [CH, D_MODEL], F16, kind="ExternalInput")
    cos_d = nc.dram_tensor("cos2", [64, SEQ], F16, kind="ExternalInput")
    sin_d = nc.dram_tensor("sin2", [64, SEQ], F16, kind="ExternalInput")
    msk_d = nc.dram_tensor("msk", [128, 384], F16, kind="ExternalInput")
    rot_d = nc.dram_tensor("rot", [128, 128], F16, kind="ExternalInput")
    idn_d = nc.dram_tensor("idn", [128, 128], F16, kind="ExternalInput")
    out_d = nc.dram_tensor("out", [SEQ, D_MODEL], F16, kind="ExternalOutput")

    NQ = SEQ // 512       # 4 q chunks of 512
    NROW = SEQ // 128     # 16 row chunks / q tiles
    KD = D_MODEL // 128   # 8 contraction chunks
    # xt tiles land in this order (k0..3 on sync queue, k4..7 on gpsimd,
    # issued in parallel) -- projection chains consume in arrival order.
    KORDER = [0, 4, 1, 5, 2, 6, 3, 7]

    with KTileContext(nc) as tc, nc.allow_low_precision(reason="fp16 pipeline"):
        with (
            tc.tile_pool(name="wgt", bufs=1) as wgt,
            tc.tile_pool(name="tabs", bufs=1) as tabs,
            tc.tile_pool(name="qk", bufs=1) as qkp,
            tc.tile_pool(name="vp", bufs=1) as vp,
            tc.tile_pool(name="xt", bufs=1) as xtp,
            tc.tile_pool(name="op", bufs=1) as op,
        ):
            wq_sb = [wgt.tile([128, CH], F16, name=f"wq{k}", tag=f"wq{k}") for k in range(KD)]
            wk_sb = [wgt.tile([128, CH], F16, name=f"wk{k}", tag=f"wk{k}") for k in range(KD)]
            wv_sb = [wgt.tile([128, CH], F16, name=f"wv{k}", tag=f"wv{k}") for k in range(KD)]
            cos_sb = tabs.tile([128, SEQ], F16, tag="cos")
            sin_sb = tabs.tile([128, SEQ], F16, tag="sin")
            rot_sb = tabs.tile([128, 128], F16, tag="rot")
            idn_sb = tabs.tile([128, 128], F16, tag="idn")
            msk_sb = tabs.tile([128, 384], F16, tag="msk")
            q_sb = [qkp.tile([128, SEQ], F16, name=f"q{t}", tag=f"q{t}") for t in range(2)]
            k_sb = [qkp.tile([128, SEQ], F16, name=f"k{t}", tag=f"k{t}") for t in range(2)]
            v_sb = [vp.tile([128, HEADS_PER_CORE, 65], F16, name=f"v{r}", tag=f"v{r}")
                    for r in range(NROW)]
            xt_sb = [xtp.tile([128, SEQ], F16, name=f"xt{k}", tag=f"xt{k}") for k in range(KD)]
            oT = [op.tile([128, SEQ], F16, name=f"oT{t}", tag=f"oT{t}") for t in range(2)]
            wo_sb = [op.tile([128, D_MODEL], F16, name=f"wo{k}", tag=f"wo{k}")
                     for k in range(2)]

            # ---- DMA issue. Weights first (small), then xt column halves
            # interleaved so chains can consume at arrival rate; tables,
            # wv and wo only after all xt (they are needed much later).
            HALF = SEQ // 2
            nc.sync.dma_start(out=wq_sb[0][:], in_=wq_d[0:128, :])
            nc.sync.dma_start(out=wq_sb[4][:], in_=wq_d[512:640, :])
            for i in range(4):
                nc.sync.dma_start(out=xt_sb[i][:, 0:HALF],
                                  in_=xt_d[i * 128:(i + 1) * 128, 0:HALF])
                if i < 3:
                    nc.sync.dma_start(out=wq_sb[i + 1][:],
                                      in_=wq_d[(i + 1) * 128:(i + 2) * 128, :])
                    nc.sync.dma_start(out=wq_sb[i + 5][:],
                                      in_=wq_d[(i + 5) * 128:(i + 6) * 128, :])
            for i in range(4):
                nc.sync.dma_start(out=xt_sb[i][:, HALF:SEQ],
                                  in_=xt_d[i * 128:(i + 1) * 128, HALF:SEQ])
            nc.gpsimd.dma_start(out=wk_sb[0][:], in_=wk_d[0:128, :])
            nc.gpsimd.dma_start(out=wk_sb[4][:], in_=wk_d[512:640, :])
            nc.gpsimd.dma_start(out=rot_sb[:], in_=rot_d[:])
            for i in range(4):
                k = i + 4
                nc.gpsimd.dma_start(out=xt_sb[k][:, 0:HALF],
                                    in_=xt_d[k * 128:(k + 1) * 128, 0:HALF])
                if i < 3:
                    nc.gpsimd.dma_start(out=wk_sb[i + 1][:],
                                        in_=wk_d[(i + 1) * 128:(i + 2) * 128, :])
                    nc.gpsimd.dma_start(out=wk_sb[i + 5][:],
                                        in_=wk_d[(i + 5) * 128:(i + 6) * 128, :])
            for i in range(4):
                k = i + 4
                nc.gpsimd.dma_start(out=xt_sb[k][:, HALF:SEQ],
                                    in_=xt_d[k * 128:(k + 1) * 128, HALF:SEQ])
            nc.gpsimd.dma_start(out=cos_sb[0:64, :], in_=cos_d[:])
            nc.gpsimd.dma_start(out=cos_sb[64:128, :], in_=cos_d[:])
            nc.gpsimd.dma_start(out=sin_sb[0:64, :], in_=sin_d[:])
            nc.gpsimd.dma_start(out=sin_sb[64:128, :], in_=sin_d[:])
            for k in range(KD):
                nc.gpsimd.dma_start(out=wv_sb[k][:], in_=wv_d[k * 128:(k + 1) * 128, :])
            nc.gpsimd.dma_start(out=idn_sb[:], in_=idn_d[:])
            nc.gpsimd.dma_start(out=msk_sb[:], in_=msk_d[:])
            for k in range(2):
                nc.gpsimd.dma_start(out=wo_sb[k][:], in_=wo_d[k * 128:(k + 1) * 128, :])

            for r in range(NROW):
                nc.vector.memset(v_sb[r][:, :, 64:65], 1.0)

            # ---------------- phase 1: projections + RoPE ----------------
            with (
                tc.tile_pool(name="praw", bufs=2) as praw,
                tc.tile_pool(name="pp", bufs=1, space="PSUM") as pp,
                tc.tile_pool(name="rp", bufs=2, space="PSUM") as rp,
            ):
                def rope(m, which, raw, dst):
                    for n in range(NQ):
                        sl = slice(n * 512, (n + 1) * 512)
                        pr = rp.tile([128, 512], F32, name=f"pr{m}{which}{n}", tag="prot")
                        nc.tensor.matmul(pr[:], rot_sb[:], raw[:, sl],
                                         start=True, stop=True)
                        t1 = praw.tile([128, 512], F16, name=f"t1{m}{which}{n}", tag="t1")
                        nc.vector.tensor_mul(t1[:], raw[:, sl], cos_sb[:, sl])
                        t2 = praw.tile([128, 512], F16, name=f"t2{m}{which}{n}", tag="t2")
                        nc.vector.tensor_mul(t2[:], pr[:], sin_sb[:, sl])
                        nc.vector.tensor_add(dst[:, sl], t1[:], t2[:])

                # --- head pair 0: two waves of 4 interleaved chains, each
                # wave consuming one xt column-half in arrival order.
                raw_q0 = praw.tile([128, SEQ], F16, name="rawq0", tag="raw")
                raw_k0 = praw.tile([128, SEQ], F16, name="rawk0", tag="raw")
                for half in range(2):
                    chains = []
                    for which in ("q", "k"):
                        for j in range(2):
                            n = 2 * half + j
                            ps = pp.tile([128, 512], F32,
                                         name=f"ps0{which}{n}", tag=f"p{which}{j}")
                            chains.append((which, n, ps))
                    for i, k in enumerate(KORDER):
                        st, sp = (i == 0), (i == KD - 1)
                        for which, n, ps in chains:
                            w_sb = wq_sb if which == "q" else wk_sb
                            nc.tensor.matmul(
                                ps[:], w_sb[k][:, 0:128],
                                xt_sb[k][:, n * 512:(n + 1) * 512],
                                start=st, stop=sp)
                    for which, n, ps in chains:
                        raw = raw_q0 if which == "q" else raw_k0
                        nc.scalar.copy(raw[:, n * 512:(n + 1) * 512], ps[:])
                rope(0, "q", raw_q0, q_sb[0])
                rope(0, "k", raw_k0, k_sb[0])

                # --- head pair 1: plain chains (xt resident)
                for which, w_sb, dst in (("q", wq_sb, q_sb), ("k", wk_sb, k_sb)):
                    raw = praw.tile([128, SEQ], F16, name=f"raw1{which}", tag="raw")
                    for n in range(NQ):
                        ps = pp.tile([128, 512], F32, name=f"ps1{which}{n}",
                                     tag=f"p{which}{n % 2}")
                        for i, k in enumerate(KORDER):
                            nc.tensor.matmul(
                                ps[:], w_sb[k][:, 128:256],
                                xt_sb[k][:, n * 512:(n + 1) * 512],
                                start=(i == 0), stop=(i == KD - 1))
                        nc.scalar.copy(raw[:, n * 512:(n + 1) * 512], ps[:])
                    rope(1, which, raw, dst[1])

                # --- v projection rows 0..3 (rest interleaved into attn)
                for r in range(4):
                    tagn = ["pq0", "pq1", "pk0", "pk1"][r % 4]
                    ps = pp.tile([128, HEADS_PER_CORE, 64], F32,
                                 name=f"pv{r}", tag=tagn)
                    for i, k in enumerate(KORDER):
                        nc.tensor.matmul(
                            ps[:].rearrange("p a b -> p (a b)"),
                            xt_sb[k][:, r * 128:(r + 1) * 128], wv_sb[k][:],
                            start=(i == 0), stop=(i == KD - 1))
                    nc.scalar.copy(v_sb[r][:, :, 0:64], ps[:])

            # ---------------- phase 2: attention + output projection ----
            with (
                tc.tile_pool(name="se", bufs=5) as sep,
                tc.tile_pool(name="nrm", bufs=2) as nrm,
                tc.tile_pool(name="onat", bufs=8) as onp,
                tc.tile_pool(name="outp", bufs=3) as outp,
                tc.tile_pool(name="pss", bufs=2, space="PSUM") as pss,
                tc.tile_pool(name="pop", bufs=2, space="PSUM") as pop,
                tc.tile_pool(name="pmisc", bufs=2, space="PSUM") as pmisc,
            ):
                # v-projection rows 4..15 as PE filler units (two halves
                # each) popped into the attention stream: they keep the PE
                # dense through the exp-bound stretches and their psum
                # shares the pmisc ring.
                vfill = []
                for r in range(4, NROW):
                    def vch(r=r):
                        ps = pmisc.tile([128, HEADS_PER_CORE, 64], F32,
                                        name=f"pv{r}", tag="pm")
                        for i, k in enumerate(KORDER):
                            nc.tensor.matmul(
                                ps[:].rearrange("p a b -> p (a b)"),
                                xt_sb[k][:, r * 128:(r + 1) * 128], wv_sb[k][:],
                                start=(i == 0), stop=(i == KD - 1))
                        nc.vector.tensor_copy(v_sb[r][:, :, 0:64], ps[:])

                    vfill.append(("v", r, vch))

                def outproj_unit(qt):
                    def emit():
                        for nn in range(2):
                            pf = pmisc.tile([128, 512], F32,
                                            name=f"pf{qt}_{nn}", tag="pm")
                            for kk in range(2):
                                nc.tensor.matmul(
                                    pf[:],
                                    oT[kk][:, qt * 128:(qt + 1) * 128],
                                    wo_sb[kk][:, nn * 512:(nn + 1) * 512],
                                    start=(kk == 0), stop=(kk == 1))
                            ob = outp.tile([128, 512], F16,
                                           name=f"ob{qt}_{nn}", tag="ob")
                            nc.vector.tensor_copy(ob[:], pf[:])
                            nc.sync.dma_start(
                                out=out_d[qt * 128:(qt + 1) * 128,
                                          nn * 512:(nn + 1) * 512],
                                in_=ob[:])
                    return emit

                def fin_piece(qc, th, po, qt4):
                    # per-q-tile finalize for the last block: po[:, qt4] is
                    # complete once av(4qc+qt4) stopped, so normalize,
                    # transpose and project it while later kts still run
                    gq = 4 * qc + qt4
                    onat_t = onp.tile([128, 2, 64], F16,
                                      name=f"onp{th}_{gq}", tag="onat")
                    for hh in range(2):
                        rcol = nrm.tile([128, 1, 1], F32,
                                        name=f"rcp{th}{gq}{hh}", tag=f"rcp{hh}")
                        nc.vector.reciprocal(rcol[:], po[hh][:, qt4:qt4 + 1, 64:65])
                        nc.vector.tensor_scalar_mul(
                            onat_t[:, hh, :], po[hh][:, qt4, 0:64],
                            rcol[:, 0, :])
                    pt = pmisc.tile([128, 128], F16, name=f"ptl{gq}", tag="pm")
                    nc.tensor.transpose(
                        pt[:], onat_t[:].rearrange("p a b -> p (a b)"), idn_sb[:])
                    nc.vector.tensor_copy(
                        oT[th][:, gq * 128:(gq + 1) * 128], pt[:])
                    outproj_unit(gq)()

                def finalize(qc, th, po, fuse_outproj=False):
                    onat_tiles = []

                    def emit_norm():
                        for qt4 in range(4):
                            onat_tiles.append(
                                onp.tile([128, 2, 64], F16,
                                         name=f"on{th}_{4 * qc + qt4}",
                                         tag="onat"))
                        rcols = []
                        for hh in range(2):
                            rcol = nrm.tile([128, 4, 1], F32,
                                            name=f"rc{th}{qc}{hh}", tag=f"rcol{hh}")
                            nc.vector.reciprocal(rcol[:], po[hh][:, :, 64:65])
                            rcols.append(rcol)
                        for qt4 in range(4):
                            for hh in range(2):
                                nc.vector.tensor_scalar_mul(
                                    onat_tiles[qt4][:, hh, :],
                                    po[hh][:, qt4, 0:64],
                                    rcols[hh][:, qt4, :])

                    def emit_tr():
                        for qt4 in range(4):
                            gq = 4 * qc + qt4
                            pt = pmisc.tile([128, 128], F16,
                                            name=f"pt{th}_{gq}", tag="pm")
                            nc.tensor.transpose(
                                pt[:],
                                onat_tiles[qt4][:].rearrange("p a b -> p (a b)"),
                                idn_sb[:])
                            nc.vector.tensor_copy(
                                oT[th][:, gq * 128:(gq + 1) * 128], pt[:])
                            if fuse_outproj:
                                outproj_unit(gq)()
                    return emit_norm, emit_tr

                # Unified PE filler queue: v-projection chains (deadline:
                # v_sb[r] must be emitted before any av that reads it, since
                # the PE executes in order) and output projections (needed
                # only before the end).  Opportunistic pops spread the units
                # through the exp-bound stretches; deadline pops guarantee
                # correctness.
                fillq = list(vfill)
                AV_LAG = 2

                def drain_vdeadline(row):
                    while fillq and fillq[0][0] == "v" and fillq[0][1] <= row:
                        fillq.pop(0)[2]()

                pend_fin = None
                for qc in range(NQ):              # q chunk of 512
                    qs0 = qc * 512
                    nkt = 4 * qc + 4              # causal k tiles
                    for th in range(2):           # head pair
                        last = (qc == NQ - 1 and th == 1)
                        po = [pop.tile([128, HEADS_PER_CORE, 65], F32,
                                       name=f"po{th}_{qc}_{hh}", tag="po")
                              for hh in range(2)]
                        av_q = []
                        for kt in range(nkt):
                            rel = kt - 4 * qc
                            c0 = max(rel, 0) * 128
                            ps = pss.tile([128, 2, 512], F32,
                                          name=f"ps{th}_{qc}_{kt}", tag="ps")
                            for hh in range(2):
                                b0 = 64 * hh
                                nc.tensor.matmul(
                                    ps[:, hh, c0:512],
                                    k_sb[th][b0:b0 + 64, kt * 128:(kt + 1) * 128],
                                    q_sb[th][b0:b0 + 64, qs0 + c0:qs0 + 512],
                                    start=True, stop=True)
                            s = sep.tile([128, 2, 512], F16,
                                         name=f"s{th}_{qc}_{kt}", tag="se")
                            nc.scalar.activation(
                                s[:, :, c0:512], ps[:, :, c0:512],
                                mybir.ActivationFunctionType.Exp, scale=0.125)
                            if rel >= 0:
                                for hh in range(2):
                                    nc.vector.tensor_mul(
                                        s[:, hh, c0:c0 + 128],
                                        s[:, hh, c0:c0 + 128],
                                        msk_sb[:, 0:128])

                            def av(kt=kt, s=s, po=po, qc=qc, th=th):
                                # first write into each po bank clears it
                                # (start=True zero-pends the whole bank;
                                # later writes then land fresh / accumulate)
                                for hh in range(2):
                                    head = 2 * th + hh
                                    for qt4 in range(4):
                                        gq = 4 * qc + qt4
                                        if kt > gq:
                                            continue
                                        nc.tensor.matmul(
                                            po[hh][:, qt4, :],
                                            s[:, hh, qt4 * 128:(qt4 + 1) * 128],
                                            v_sb[kt][:, head, :],
                                            start=(kt == 0 and qt4 == 0),
                                            stop=(kt == gq),
                                            skip_group_check=True)

                            if kt == 1 and pend_fin is not None:
                                # DVE normalize first, a PE filler to cover
                                # its latency, then the PE transposes
                                pend_fin[0]()
                                # at th0 blocks the pending fin is th1's:
                                # an outproj unit here could read oT columns
                                # the transpose below hasn't written yet
                                if fillq and (fillq[0][0] == "v" or th == 1):
                                    fillq.pop(0)[2]()
                                pend_fin[1]()
                                pend_fin = None
                            drain_vdeadline(4 * qc + kt + 2)
                            if kt >= 2 and kt % 3 != 2 and fillq:
                                fillq.pop(0)[2]()
                            # attn@v lags one kt behind its scores so it
                            # never waits on the exp/mask chain
                            if len(av_q) == AV_LAG:
                                av_q.pop(0)()
                                if last and kt - AV_LAG >= 4 * qc:
                                    fin_piece(qc, th, po, kt - AV_LAG - 4 * qc)
                            av_q.append(av)
                        for i, a in enumerate(av_q):
                            a()
                            if last and nkt - len(av_q) + i >= 4 * qc:
                                fin_piece(qc, th, po, nkt - len(av_q) + i - 4 * qc)
                        av_q = []
                        if not last:
                            pend_fin = finalize(qc, th, po)
                        if th == 1 and not last:
                            for qt4 in range(4):
                                fillq.append(("o", 10 ** 6,
                                              outproj_unit(4 * qc + qt4)))
                if pend_fin is not None:
                    pend_fin[0]()
                    pend_fin[1]()
                for u in fillq:
                    u[2]()
    return nc


_PROGRAM_CACHE = {}


def _get_program():
    if "nc" not in _PROGRAM_CACHE:
        _PROGRAM_CACHE["nc"] = build_program()
    return _PROGRAM_CACHE["nc"]


def _host_inputs(x, cos, sin, Wq, Wk, Wv, Wo):
    f16 = np.float16
    cosT = np.ascontiguousarray(cos.T).astype(f16)
    sinT = np.ascontiguousarray(sin.T).astype(f16)

    R = np.zeros((HEAD_DIM, HEAD_DIM), np.float32)
    R[np.arange(32), np.arange(32) + 32] = -1.0
    R[np.arange(32) + 32, np.arange(32)] = 1.0
    RT = R.T
    rot = np.zeros((128, 128), np.float32)
    rot[0:64, 0:64] = RT
    rot[64:128, 64:128] = RT
    rot = rot.astype(f16)

    msk = np.zeros((128, 384), np.float32)
    p = np.arange(128)[:, None]
    f = np.arange(128)[None, :]
    msk[:, 0:128] = (p - f <= 0)          # triangular block; cols 128: zeros
    msk = msk.astype(f16)

    idn = np.eye(128, dtype=f16)

    in_maps = []
    for c in range(N_CORES):
        b, g = divmod(c, GROUPS)
        rows = slice(g * CH, (g + 1) * CH)
        in_maps.append({
            "xt": np.ascontiguousarray(x[b].T).astype(f16),
            "wq": np.ascontiguousarray(Wq[rows, :].T).astype(f16),
            "wk": np.ascontiguousarray(Wk[rows, :].T).astype(f16),
            "wv": np.ascontiguousarray(Wv[rows, :].T).astype(f16),
            "wo": np.ascontiguousarray(Wo[:, rows].T).astype(f16),
            "cos2": cosT, "sin2": sinT, "msk": msk, "rot": rot, "idn": idn,
        })
    return in_maps


def kernel(x, cos, sin, Wq, Wk, Wv, Wo, _trace=False, _trace_kwargs=None):
    nc = _get_program()
    in_maps = _host_inputs(x, cos, sin, Wq, Wk, Wv, Wo)
    kw = {}
    if _trace:
        kw["trace"] = True
        if _trace_kwargs:
            kw.update(_trace_kwargs)
    res = run_bass_kernel_spmd(nc, in_maps, list(range(N_CORES)), **kw)
    out = np.zeros((BATCH, SEQ, D_MODEL), np.float32)
    for c in range(N_CORES):
        b = c // GROUPS
        out[b] += res.results[c]["out"].astype(np.float32)
    kernel.last_result = res
    return out


# revision 36
# speedup vs baseline: 1.2450x; 1.0104x over previous
"""Causal self-attention (RoPE, 16 heads) on 8 Trainium2 NeuronCores.

Sharding: data parallel over batch (2) x tensor parallel over head groups
(16 heads -> 4 groups of 4). Core c handles batch c//4, head group c%4.
Each core computes q/k/v projections for its 4 heads, RoPE, causal
softmax(q k^T / sqrt(d)) v, and its slice of the output projection; the
host sums the 4 tensor-parallel partials per batch.

Layouts (per core):
  xT [1024 D, 2048 S]   q/k transposed [256 ch, 2048 S] (head dim on
  partitions, so scores need no transposes), v natural [2048 S, 4, 64+1]
  with a ones column so attn@v also produces the softmax denominators.
  Scores are computed transposed S[k, q]; attn@v uses the exp tiles as
  the stationary operand giving o in natural [q, ch] layout, where the
  denominator lands in a psum column -> per-partition reciprocal +
  tensor_scalar normalize. o is then PE-transposed back to [ch, q] for
  the output projection. fp16 operands, fp32 psum accumulation.

Schedule: weights are DMAd first (small), then xt arrives in column
halves interleaved across two issue queues at the same rate the
projection chains consume them, so the PE starts ~7.5us in and stays
dense. Attention runs qc-outer/th-inner; each block's first attn@v and
the previous block's normalize/transpose/output-projection are deferred
into the next block's score stream so the PE never waits on the Vector
engine at block boundaries. PSUM accumulators are zeroed through the
bank's start=True write semantics (first write per bank clears it).
"""
import numpy as np

import concourse.bass as bass
import concourse.mybir as mybir
import concourse.tile as tile
from concourse.vector_clock import ScopedClock
from concourse.bass_utils import run_bass_kernel_spmd

F32 = mybir.dt.float32
F16 = mybir.dt.float16

D_MODEL = 1024
N_HEADS = 16
HEAD_DIM = 64
SEQ = 2048
BATCH = 2
N_CORES = 8
HEADS_PER_CORE = 4
GROUPS = 4
CH = HEADS_PER_CORE * HEAD_DIM  # 256

MAX_WAITS = 1


def _cap_waits(nc: bass.Bass, cap: int):
    """walrus here only accepts `cap` sem waits per instruction; hoist the
    overflow onto same-engine nops inserted just before."""
    nid = [0]

    def mknop(engine, waits):
        nid[0] += 1
        n = mybir.InstNoOp(name=f"I-waitcap-{nid[0]}", ins=[], outs=[])
        n.engine = engine
        n.sync_info = mybir.SyncInfo(on_wait=list(waits), on_update=[])
        return n

    for fn in nc.m.functions:
        for bb in fn.blocks:
            out = []
            changed = False
            for ins in bb.instructions:
                si = ins.sync_info
                w = list(si.on_wait) if si and si.on_wait else []
                if len(w) > cap:
                    changed = True
                    keep = w[-cap:]
                    rest = w[: len(w) - cap]
                    eng = ins.engine
                    if eng == mybir.EngineType.Unassigned:
                        eng = mybir.EngineType.SP
                    for i in range(0, len(rest), cap):
                        out.append(mknop(eng, rest[i : i + cap]))
                    si.on_wait = keep
                out.append(ins)
            if changed:
                bb.instructions = out


class KTileContext(tile.TileContext):
    def _drain_and_barrier(self, tick_clock, wait_clock):
        drain_inst = self.nc.sync.drain()
        wait_clock.add_sem_waits(
            drain_inst.ins, ScopedClock({None: tick_clock.global_clock})
        )
        si = drain_inst.ins.sync_info
        w = si.on_wait if si else None
        if w and len(w) > 1:
            si.on_wait = []
            for sw in w:
                n2 = self.nc.sync.nop()
                if n2.ins.sync_info is None:
                    n2.ins.sync_info = mybir.SyncInfo(on_wait=[sw], on_update=[])
                else:
                    n2.ins.sync_info.on_wait = [sw]
            self.nc.sync.drain()
        self.nc.all_engine_barrier()
        assert self.sems is not None
        popped = self.nc._tile_sem_poison_stack.pop()
        assert popped is self._sem_poison
        self.nc.clear_and_free_semaphores(list(self.sems.allocated().values()))
        self.nc.all_engine_barrier()

    def __exit__(self, exc_type, exc_value, traceback):
        r = super().__exit__(exc_type, exc_value, traceback)
        if exc_type is None:
            _cap_waits(self.nc, MAX_WAITS)
        return r


def build_program() -> bass.Bass:
    nc = bass.Bass()

    xt_d = nc.dram_tensor("xt", [D_MODEL, SEQ], F16, kind="ExternalInput")
    wq_d = nc.dram_tensor("wq", [D_MODEL, CH], F16, kind="ExternalInput")
    wk_d = nc.dram_tensor("wk", [D_MODEL, CH], F16, kind="ExternalInput")
    wv_d = nc.dram_tensor("wv", [D_MODEL, CH], F16, kind="ExternalInput")
    wo_d = nc.dram_tensor("wo", [CH, D_MODEL], F16, kind="ExternalInput")
    cos_d = nc.dram_tensor("cos2", [64, SEQ], F16, kind="ExternalInput")
    sin_d = nc.dram_tensor("sin2", [64, SEQ], F16, kind="ExternalInput")
    msk_d = nc.dram_tensor("msk", [128, 384], F16, kind="ExternalInput")
    rot_d = nc.dram_tensor("rot", [128, 128], F16, kind="ExternalInput")
    idn_d = nc.dram_tensor("idn", [128, 128], F16, kind="ExternalInput")
    out_d = nc.dram_tensor("out", [SEQ, D_MODEL], F16, kind="ExternalOutput")

    NQ = SEQ // 512       # 4 q chunks of 512
    NROW = SEQ // 128     # 16 row chunks / q tiles
    KD = D_MODEL // 128   # 8 contraction chunks
    # xt tiles land in this order (k0..3 on sync queue, k4..7 on gpsimd,
    # issued in parallel) -- projection chains consume in arrival order.
    KORDER = [0, 4, 1, 5, 2, 6, 3, 7]

    with KTileContext(nc) as tc, nc.allow_low_precision(reason="fp16 pipeline"):
        with (
            tc.tile_pool(name="wgt", bufs=1) as wgt,
            tc.tile_pool(name="tabs", bufs=1) as tabs,
            tc.tile_pool(name="qk", bufs=1) as qkp,
            tc.tile_pool(name="vp", bufs=1) as vp,
            tc.tile_pool(name="xt", bufs=1) as xtp,
            tc.tile_pool(name="op", bufs=1) as op,
        ):
            wq_sb = [wgt.tile([128, CH], F16, name=f"wq{k}", tag=f"wq{k}") for k in range(KD)]
            wk_sb = [wgt.tile([128, CH], F16, name=f"wk{k}", tag=f"wk{k}") for k in range(KD)]
            wv_sb = [wgt.tile([128, CH], F16, name=f"wv{k}", tag=f"wv{k}") for k in range(KD)]
            cos_sb = tabs.tile([128, SEQ], F16, tag="cos")
            sin_sb = tabs.tile([128, SEQ], F16, tag="sin")
            rot_sb = tabs.tile([128, 128], F16, tag="rot")
            idn_sb = tabs.tile([128, 128], F16, tag="idn")
            msk_sb = tabs.tile([128, 384], F16, tag="msk")
            q_sb = [qkp.tile([128, SEQ], F16, name=f"q{t}", tag=f"q{t}") for t in range(2)]
            k_sb = [qkp.tile([128, SEQ], F16, name=f"k{t}", tag=f"k{t}") for t in range(2)]
            v_sb = [vp.tile([128, HEADS_PER_CORE, 65], F16, name=f"v{r}", tag=f"v{r}")
                    for r in range(NROW)]
            xt_sb = [xtp.tile([128, SEQ], F16, name=f"xt{k}", tag=f"xt{k}") for k in range(KD)]
            oT = [op.tile([128, SEQ], F16, name=f"oT{t}", tag=f"oT{t}") for t in range(2)]
            wo_sb = [op.tile([128, D_MODEL], F16, name=f"wo{k}", tag=f"wo{k}")
                     for k in range(2)]

            # ---- DMA issue. Weights first (small), then xt column halves
            # interleaved so chains can consume at arrival rate; tables,
            # wv and wo only after all xt (they are needed much later).
            HALF = SEQ // 2
            nc.sync.dma_start(out=wq_sb[0][:], in_=wq_d[0:128, :])
            nc.sync.dma_start(out=wq_sb[4][:], in_=wq_d[512:640, :])
            for i in range(4):
                nc.sync.dma_start(out=xt_sb[i][:, 0:HALF],
                                  in_=xt_d[i * 128:(i + 1) * 128, 0:HALF])
                if i < 3:
                    nc.sync.dma_start(out=wq_sb[i + 1][:],
                                      in_=wq_d[(i + 1) * 128:(i + 2) * 128, :])
                    nc.sync.dma_start(out=wq_sb[i + 5][:],
                                      in_=wq_d[(i + 5) * 128:(i + 6) * 128, :])
            for i in range(4):
                nc.sync.dma_start(out=xt_sb[i][:, HALF:SEQ],
                                  in_=xt_d[i * 128:(i + 1) * 128, HALF:SEQ])
            nc.gpsimd.dma_start(out=wk_sb[0][:], in_=wk_d[0:128, :])
            nc.gpsimd.dma_start(out=wk_sb[4][:], in_=wk_d[512:640, :])
            nc.gpsimd.dma_start(out=rot_sb[:], in_=rot_d[:])
            for i in range(4):
                k = i + 4
                nc.gpsimd.dma_start(out=xt_sb[k][:, 0:HALF],
                                    in_=xt_d[k * 128:(k + 1) * 128, 0:HALF])
                if i < 3:
                    nc.gpsimd.dma_start(out=wk_sb[i + 1][:],
                                        in_=wk_d[(i + 1) * 128:(i + 2) * 128, :])
                    nc.gpsimd.dma_start(out=wk_sb[i + 5][:],
                                        in_=wk_d[(i + 5) * 128:(i + 6) * 128, :])
            for i in range(4):
                k = i + 4
                nc.gpsimd.dma_start(out=xt_sb[k][:, HALF:SEQ],
                                    in_=xt_d[k * 128:(k + 1) * 128, HALF:SEQ])
            nc.gpsimd.dma_start(out=cos_sb[0:64, :], in_=cos_d[:])
            nc.gpsimd.dma_start(out=cos_sb[64:128, :], in_=cos_d[:])
            nc.gpsimd.dma_start(out=sin_sb[0:64, :], in_=sin_d[:])
            nc.gpsimd.dma_start(out=sin_sb[64:128, :], in_=sin_d[:])
            for k in range(KD):
                nc.gpsimd.dma_start(out=wv_sb[k][:], in_=wv_d[k * 128:(k + 1) * 128, :])
            nc.gpsimd.dma_start(out=idn_sb[:], in_=idn_d[:])
            nc.gpsimd.dma_start(out=msk_sb[:], in_=msk_d[:])
            for k in range(2):
                nc.gpsimd.dma_start(out=wo_sb[k][:], in_=wo_d[k * 128:(k + 1) * 128, :])

            for r in range(NROW):
                nc.vector.memset(v_sb[r][:, :, 64:65], 1.0)

            # ---------------- phase 1: projections + RoPE ----------------
            with (
                tc.tile_pool(name="praw", bufs=2) as praw,
                tc.tile_pool(name="pp", bufs=1, space="PSUM") as pp,
                tc.tile_pool(name="rp", bufs=2, space="PSUM") as rp,
            ):
                def rope(m, which, raw, dst):
                    for n in range(NQ):
                        sl = slice(n * 512, (n + 1) * 512)
                        pr = rp.tile([128, 512], F32, name=f"pr{m}{which}{n}", tag="prot")
                        nc.tensor.matmul(pr[:], rot_sb[:], raw[:, sl],
                                         start=True, stop=True)
                        t1 = praw.tile([128, 512], F16, name=f"t1{m}{which}{n}", tag="t1")
                        nc.vector.tensor_mul(t1[:], raw[:, sl], cos_sb[:, sl])
                        t2 = praw.tile([128, 512], F16, name=f"t2{m}{which}{n}", tag="t2")
                        nc.vector.tensor_mul(t2[:], pr[:], sin_sb[:, sl])
                        nc.vector.tensor_add(dst[:, sl], t1[:], t2[:])

                # --- head pair 0: two waves of 4 interleaved chains, each
                # wave consuming one xt column-half in arrival order.
                raw_q0 = praw.tile([128, SEQ], F16, name="rawq0", tag="raw")
                raw_k0 = praw.tile([128, SEQ], F16, name="rawk0", tag="raw")
                for half in range(2):
                    chains = []
                    for which in ("q", "k"):
                        for j in range(2):
                            n = 2 * half + j
                            ps = pp.tile([128, 512], F32,
                                         name=f"ps0{which}{n}", tag=f"p{which}{j}")
                            chains.append((which, n, ps))
                    for i, k in enumerate(KORDER):
                        st, sp = (i == 0), (i == KD - 1)
                        for which, n, ps in chains:
                            w_sb = wq_sb if which == "q" else wk_sb
                            nc.tensor.matmul(
                                ps[:], w_sb[k][:, 0:128],
                                xt_sb[k][:, n * 512:(n + 1) * 512],
                                start=st, stop=sp)
                    for which, n, ps in chains:
                        raw = raw_q0 if which == "q" else raw_k0
                        nc.scalar.copy(raw[:, n * 512:(n + 1) * 512], ps[:])
                rope(0, "q", raw_q0, q_sb[0])
                rope(0, "k", raw_k0, k_sb[0])

                # --- head pair 1: plain chains (xt resident)
                for which, w_sb, dst in (("q", wq_sb, q_sb), ("k", wk_sb, k_sb)):
                    raw = praw.tile([128, SEQ], F16, name=f"raw1{which}", tag="raw")
                    for n in range(NQ):
                        ps = pp.tile([128, 512], F32, name=f"ps1{which}{n}",
                                     tag=f"p{which}{n % 2}")
                        for i, k in enumerate(KORDER):
                            nc.tensor.matmul(
                                ps[:], w_sb[k][:, 128:256],
                                xt_sb[k][:, n * 512:(n + 1) * 512],
                                start=(i == 0), stop=(i == KD - 1))
                        nc.scalar.copy(raw[:, n * 512:(n + 1) * 512], ps[:])
                    rope(1, which, raw, dst[1])

                # --- v projection rows 0..3 (rest interleaved into attn)
                for r in range(4):
                    tagn = ["pq0", "pq1", "pk0", "pk1"][r % 4]
                    ps = pp.tile([128, HEADS_PER_CORE, 64], F32,
                                 name=f"pv{r}", tag=tagn)
                    for i, k in enumerate(KORDER):
                        nc.tensor.matmul(
                            ps[:].rearrange("p a b -> p (a b)"),
                            xt_sb[k][:, r * 128:(r + 1) * 128], wv_sb[k][:],
                            start=(i == 0), stop=(i == KD - 1))
                    nc.scalar.copy(v_sb[r][:, :, 0:64], ps[:])

            # ---------------- phase 2: attention + output projection ----
            with (
                tc.tile_pool(name="se", bufs=6) as sep,
                tc.tile_pool(name="nrm", bufs=2) as nrm,
                tc.tile_pool(name="onat", bufs=8) as onp,
                tc.tile_pool(name="outp", bufs=3) as outp,
                tc.tile_pool(name="pss", bufs=2, space="PSUM") as pss,
                tc.tile_pool(name="pop", bufs=2, space="PSUM") as pop,
                tc.tile_pool(name="pmisc", bufs=2, space="PSUM") as pmisc,
            ):
                # v-projection rows 4..15 as PE filler units (two halves
                # each) popped into the attention stream: they keep the PE
                # dense through the exp-bound stretches and their psum
                # shares the pmisc ring.
                vfill = []
                for r in range(4, NROW):
                    def vch(r=r):
                        ps = pmisc.tile([128, HEADS_PER_CORE, 64], F32,
                                        name=f"pv{r}", tag="pm")
                        for i, k in enumerate(KORDER):
                            nc.tensor.matmul(
                                ps[:].rearrange("p a b -> p (a b)"),
                                xt_sb[k][:, r * 128:(r + 1) * 128], wv_sb[k][:],
                                start=(i == 0), stop=(i == KD - 1))
                        nc.vector.tensor_copy(v_sb[r][:, :, 0:64], ps[:])

                    vfill.append(("v", r, vch))

                def outproj_unit(qt):
                    def emit():
                        for nn in range(2):
                            pf = pmisc.tile([128, 512], F32,
                                            name=f"pf{qt}_{nn}", tag="pm")
                            for kk in range(2):
                                nc.tensor.matmul(
                                    pf[:],
                                    oT[kk][:, qt * 128:(qt + 1) * 128],
                                    wo_sb[kk][:, nn * 512:(nn + 1) * 512],
                                    start=(kk == 0), stop=(kk == 1))
                            ob = outp.tile([128, 512], F16,
                                           name=f"ob{qt}_{nn}", tag="ob")
                            nc.vector.tensor_copy(ob[:], pf[:])
                            nc.sync.dma_start(
                                out=out_d[qt * 128:(qt + 1) * 128,
                                          nn * 512:(nn + 1) * 512],
                                in_=ob[:])
                    return emit

                def fin_piece(qc, th, po, qt4):
                    # per-q-tile finalize for the last block: po[:, qt4] is
                    # complete once av(4qc+qt4) stopped, so normalize,
                    # transpose and project it while later kts still run
                    gq = 4 * qc + qt4
                    onat_t = onp.tile([128, 2, 64], F16,
                                      name=f"onp{th}_{gq}", tag="onat")
                    for hh in range(2):
                        rcol = nrm.tile([128, 1, 1], F32,
                                        name=f"rcp{th}{gq}{hh}", tag=f"rcp{hh}")
                        nc.vector.reciprocal(rcol[:], po[hh][:, qt4:qt4 + 1, 64:65])
                        nc.vector.tensor_scalar_mul(
                            onat_t[:, hh, :], po[hh][:, qt4, 0:64],
                            rcol[:, 0, :])
                    pt = pmisc.tile([128, 128], F16, name=f"ptl{gq}", tag="pm")
                    nc.tensor.transpose(
                        pt[:], onat_t[:].rearrange("p a b -> p (a b)"), idn_sb[:])
                    nc.vector.tensor_copy(
                        oT[th][:, gq * 128:(gq + 1) * 128], pt[:])
                    outproj_unit(gq)()

                def finalize(qc, th, po, fuse_outproj=False):
                    onat_tiles = []

                    def emit_norm():
                        for qt4 in range(4):
                            onat_tiles.append(
                                onp.tile([128, 2, 64], F16,
                                         name=f"on{th}_{4 * qc + qt4}",
                                         tag="onat"))
                        rcols = []
                        for hh in range(2):
                            rcol = nrm.tile([128, 4, 1], F32,
                                            name=f"rc{th}{qc}{hh}", tag=f"rcol{hh}")
                            nc.vector.reciprocal(rcol[:], po[hh][:, :, 64:65])
                            rcols.append(rcol)
                        for qt4 in range(4):
                            for hh in range(2):
                                nc.vector.tensor_scalar_mul(
                                    onat_tiles[qt4][:, hh, :],
                                    po[hh][:, qt4, 0:64],
                                    rcols[hh][:, qt4, :])

                    def emit_tr():
                        for qt4 in range(4):
                            gq = 4 * qc + qt4
                            pt = pmisc.tile([128, 128], F16,
                                            name=f"pt{th}_{gq}", tag="pm")
                            nc.tensor.transpose(
                                pt[:],
                                onat_tiles[qt4][:].rearrange("p a b -> p (a b)"),
                                idn_sb[:])
                            nc.vector.tensor_copy(
                                oT[th][:, gq * 128:(gq + 1) * 128], pt[:])
                            if fuse_outproj:
                                outproj_unit(gq)()
                    return emit_norm, emit_tr

                # Unified PE filler queue: v-projection chains (deadline:
                # v_sb[r] must be emitted before any av that reads it, since
                # the PE executes in order) and output projections (needed
                # only before the end).  Opportunistic pops spread the units
                # through the exp-bound stretches; deadline pops guarantee
                # correctness.
                fillq = list(vfill)
                AV_LAG = 3

                def drain_vdeadline(row):
                    while fillq and fillq[0][0] == "v" and fillq[0][1] <= row:
                        fillq.pop(0)[2]()

                pend_fin = None
                for qc in range(NQ):              # q chunk of 512
                    qs0 = qc * 512
                    nkt = 4 * qc + 4              # causal k tiles
                    for th in range(2):           # head pair
                        last = (qc == NQ - 1 and th == 1)
                        po = [pop.tile([128, HEADS_PER_CORE, 65], F32,
                                       name=f"po{th}_{qc}_{hh}", tag="po")
                              for hh in range(2)]
                        av_q = []
                        for kt in range(nkt):
                            rel = kt - 4 * qc
                            c0 = max(rel, 0) * 128
                            ps = pss.tile([128, 2, 512], F32,
                                          name=f"ps{th}_{qc}_{kt}", tag="ps")
                            for hh in range(2):
                                b0 = 64 * hh
                                nc.tensor.matmul(
                                    ps[:, hh, c0:512],
                                    k_sb[th][b0:b0 + 64, kt * 128:(kt + 1) * 128],
                                    q_sb[th][b0:b0 + 64, qs0 + c0:qs0 + 512],
                                    start=True, stop=True)
                            s = sep.tile([128, 2, 512], F16,
                                         name=f"s{th}_{qc}_{kt}", tag="se")
                            nc.scalar.activation(
                                s[:, :, c0:512], ps[:, :, c0:512],
                                mybir.ActivationFunctionType.Exp, scale=0.125)
                            if rel >= 0:
                                for hh in range(2):
                                    nc.vector.tensor_mul(
                                        s[:, hh, c0:c0 + 128],
                                        s[:, hh, c0:c0 + 128],
                                        msk_sb[:, 0:128])

                            def av(kt=kt, s=s, po=po, qc=qc, th=th):
                                # first write into each po bank clears it
                                # (start=True zero-pends the whole bank;
                                # later writes then land fresh / accumulate)
                                for hh in range(2):
                                    head = 2 * th + hh
                                    for qt4 in range(4):
                                        gq = 4 * qc + qt4
                                        if kt > gq:
                                            continue
                                        nc.tensor.matmul(
                                            po[hh][:, qt4, :],
                                            s[:, hh, qt4 * 128:(qt4 + 1) * 128],
                                            v_sb[kt][:, head, :],
                                            start=(kt == 0 and qt4 == 0),
                                            stop=(kt == gq),
                                            skip_group_check=True)

                            if kt == 1 and pend_fin is not None:
                                # DVE normalize first, a PE filler to cover
                                # its latency, then the PE transposes
                                pend_fin[0]()
                                # at th0 blocks the pending fin is th1's:
                                # an outproj unit here could read oT columns
                                # the transpose below hasn't written yet
                                if fillq and (fillq[0][0] == "v" or th == 1):
                                    fillq.pop(0)[2]()
                                pend_fin[1]()
                                pend_fin = None
                            drain_vdeadline(4 * qc + kt + 2)
                            if kt >= 2 and kt % 3 != 2 and fillq:
                                fillq.pop(0)[2]()
                            # attn@v lags one kt behind its scores so it
                            # never waits on the exp/mask chain
                            if len(av_q) == AV_LAG:
                                av_q.pop(0)()
                                if last and kt - AV_LAG >= 4 * qc:
                                    fin_piece(qc, th, po, kt - AV_LAG - 4 * qc)
                            av_q.append(av)
                        for i, a in enumerate(av_q):
                            a()
                            if last and nkt - len(av_q) + i >= 4 * qc:
                                fin_piece(qc, th, po, nkt - len(av_q) + i - 4 * qc)
                        av_q = []
                        if not last:
                            pend_fin = finalize(qc, th, po)
                        if th == 1 and not last:
                            for qt4 in range(4):
                                fillq.append(("o", 10 ** 6,
                                              outproj_unit(4 * qc + qt4)))
                if pend_fin is not None:
                    pend_fin[0]()
                    pend_fin[1]()
                for u in fillq:
                    u[2]()
    return nc


_PROGRAM_CACHE = {}


def _get_program():
    if "nc" not in _PROGRAM_CACHE:
        _PROGRAM_CACHE["nc"] = build_program()
    return _PROGRAM_CACHE["nc"]


def _host_inputs(x, cos, sin, Wq, Wk, Wv, Wo):
    f16 = np.float16
    cosT = np.ascontiguousarray(cos.T).astype(f16)
    sinT = np.ascontiguousarray(sin.T).astype(f16)

    R = np.zeros((HEAD_DIM, HEAD_DIM), np.float32)
    R[np.arange(32), np.arange(32) + 32] = -1.0
    R[np.arange(32) + 32, np.arange(32)] = 1.0
    RT = R.T
    rot = np.zeros((128, 128), np.float32)
    rot[0:64, 0:64] = RT
    rot[64:128, 64:128] = RT
    rot = rot.astype(f16)

    msk = np.zeros((128, 384), np.float32)
    p = np.arange(128)[:, None]
    f = np.arange(128)[None, :]
    msk[:, 0:128] = (p - f <= 0)          # triangular block; cols 128: zeros
    msk = msk.astype(f16)

    idn = np.eye(128, dtype=f16)

    in_maps = []
    for c in range(N_CORES):
        b, g = divmod(c, GROUPS)
        rows = slice(g * CH, (g + 1) * CH)
        in_maps.append({
            "xt": np.ascontiguousarray(x[b].T).astype(f16),
            "wq": np.ascontiguousarray(Wq[rows, :].T).astype(f16),
            "wk": np.ascontiguousarray(Wk[rows, :].T).astype(f16),
            "wv": np.ascontiguousarray(Wv[rows, :].T).astype(f16),
            "wo": np.ascontiguousarray(Wo[:, rows].T).astype(f16),
            "cos2": cosT, "sin2": sinT, "msk": msk, "rot": rot, "idn": idn,
        })
    return in_maps


def kernel(x, cos, sin, Wq, Wk, Wv, Wo, _trace=False, _trace_kwargs=None):
    nc = _get_program()
    in_maps = _host_inputs(x, cos, sin, Wq, Wk, Wv, Wo)
    kw = {}
    if _trace:
        kw["trace"] = True
        if _trace_kwargs:
            kw.update(_trace_kwargs)
    res = run_bass_kernel_spmd(nc, in_maps, list(range(N_CORES)), **kw)
    out = np.zeros((BATCH, SEQ, D_MODEL), np.float32)
    for c in range(N_CORES):
        b = c // GROUPS
        out[b] += res.results[c]["out"].astype(np.float32)
    kernel.last_result = res
    return out
